# revision 45
# baseline (speedup 1.0000x reference)
"""Trainium2 Bass kernel for nn_LogicalReasoningLayer (moe_routing).

Sharding: 8 cores <- (batch b = c//2, seq half = c%2), 1024 tokens each.
Attention K/V exchanged between seq-half pairs via AllGather (groups of 2);
the gathered slots are already in canonical order, so both halves are read
straight out of k_recv/v_recv (no parity blending).

Activations feature-major [128p, C=4 chunks, T tokens]. Stationary matmul
operands (weights) in bf16; moving operands fp32r/bf16 (full-rate PE).
Attention AV + softmax denominator use fp8e4 DoubleRow matmuls (2 k-tiles
per instruction at 0.5 cycles/row).
"""

import sys

sys.path.insert(0, "/opt/trn_rl_repo")

import math

import ml_dtypes
import numpy as np

import concourse.bass as bass
import concourse.bacc as bacc
import concourse.tile as tile
from concourse import mybir
from concourse.bass import ts
from concourse.bass_utils import run_bass_kernel_spmd
from concourse.masks import make_identity

P = 128
H = 512
C = H // P          # 4 feature chunks
T = 1024            # tokens per core
TT = T // P         # 8 token tiles
TC = T // 512       # 2 token chunks (moving dim 512)
O = 6
NH = 4
HD = 128
D = 3
S = 2048
KT = S // P         # 16 key tiles
EPS = 1e-5
F32 = mybir.dt.float32
F32R = mybir.dt.float32r
BF16 = mybir.dt.bfloat16
FP8 = mybir.dt.float8e4
AF = mybir.ActivationFunctionType
ALU = mybir.AluOpType
DR = mybir.MatmulPerfMode.DoubleRow
RG = [[0, 1], [2, 3], [4, 5], [6, 7]]

_CACHE = {}
PHASES = []


def _mark(nc, label):
    PHASES.append((label, nc.get_next_instruction_name()))


def bcast_ap(handle, n_free):
    """[n_free] DRAM vector -> [P, n_free] stride-0 partition-broadcast AP."""
    return bass.AP(tensor=handle, offset=0, ap=[[0, P], [1, n_free]])


def build_bass(sim_mode=False):
    nc = bacc.Bacc("TRN2", target_bir_lowering=False, num_devices=8)

    f = F32
    fr = F32R
    x_in = nc.dram_tensor("x", [T, H], f, kind="ExternalInput")
    onesbf_in = nc.dram_tensor("onesbf_in", [P, P], BF16, kind="ExternalInput")
    onesfr_in = nc.dram_tensor("onesfr_in", [P, P], F32R, kind="ExternalInput")
    ones8_in = nc.dram_tensor("ones8_in", [P, 2, P], FP8, kind="ExternalInput")
    selW1T = nc.dram_tensor("selW1T", [P, C, H], BF16, kind="ExternalInput")
    selW2a = nc.dram_tensor("selW2a", [P, C, O], BF16, kind="ExternalInput")
    selb1 = nc.dram_tensor("selb1", [P, C], f, kind="ExternalInput")
    selb2 = nc.dram_tensor("selb2", [O], f, kind="ExternalInput")
    w1xT = nc.dram_tensor("w1xT", [O, P, C, H], BF16, kind="ExternalInput")
    w2T = nc.dram_tensor("w2T", [O, P, C, H], BF16, kind="ExternalInput")
    c1_d = nc.dram_tensor("c1", [P, O, C], f, kind="ExternalInput")
    lng_d = nc.dram_tensor("lng", [P, O, C], f, kind="ExternalInput")
    lnb_d = nc.dram_tensor("lnb", [P, O, C], f, kind="ExternalInput")
    inwqT = nc.dram_tensor("inwqT", [P, C, H], BF16, kind="ExternalInput")
    inwkT = nc.dram_tensor("inwkT", [P, C, H], BF16, kind="ExternalInput")
    inwvT = nc.dram_tensor("inwvT", [P, C, H], BF16, kind="ExternalInput")
    inbqk = nc.dram_tensor("inbqk", [P, 8], f, kind="ExternalInput")
    inbv = nc.dram_tensor("inbv", [H], f, kind="ExternalInput")
    outwT = nc.dram_tensor("outwT", [P, C, H], BF16, kind="ExternalInput")
    outb_a = nc.dram_tensor("outb_a", [P, C], f, kind="ExternalInput")
    rpjT = nc.dram_tensor("rpjT", [P, C, H], BF16, kind="ExternalInput")
    rpjb = nc.dram_tensor("rpjb", [P, C], f, kind="ExternalInput")
    waT = nc.dram_tensor("waT", [P, C, H], BF16, kind="ExternalInput")
    dc_d = nc.dram_tensor("dc", [P, D, C], f, kind="ExternalInput")
    gw1T = nc.dram_tensor("gw1T", [P, C, H], BF16, kind="ExternalInput")
    gw2T = nc.dram_tensor("gw2T", [P, C, H], BF16, kind="ExternalInput")
    gateb = nc.dram_tensor("gateb", [P, C], f, kind="ExternalInput")
    outwF = nc.dram_tensor("outwF", [P, C, H], BF16, kind="ExternalInput")
    outbF = nc.dram_tensor("outbF", [H], f, kind="ExternalInput")

    out_d = nc.dram_tensor("out", [T, H], f, kind="ExternalOutput")

    opw_fea = nc.dram_tensor("opw_fea", [O, T], BF16)
    k_send = nc.dram_tensor("k_send", [TC, P, NH, 512], BF16)
    v_send = nc.dram_tensor("v_send", [TC, P, 4, H], FP8)
    k_recv = nc.dram_tensor("k_recv", [TC, 2, P, NH, 512], BF16)
    v_recv = nc.dram_tensor("v_recv", [TC, 2, P, 4, H], FP8)

    with tile.TileContext(nc) as tc:
        with (
            tc.tile_pool(name="singles", bufs=1) as sg,
            tc.tile_pool(name="act", bufs=1) as ap_,
            tc.tile_pool(name="big", bufs=2) as big,
            tc.tile_pool(name="med", bufs=2) as med,
            tc.tile_pool(name="wstr", bufs=5) as wstr,
            tc.tile_pool(name="t512", bufs=4) as tp,
            tc.tile_pool(name="st", bufs=5) as st,
            tc.tile_pool(name="prep", bufs=6) as prp,
            tc.tile_pool(name="hp", bufs=2) as hp,
            tc.tile_pool(name="wrp", bufs=2) as wrpp,
            tc.tile_pool(name="kloc", bufs=4) as klp,
            tc.tile_pool(name="exp", bufs=4) as xp,
            tc.tile_pool(name="sm", bufs=4) as smp,
            tc.tile_pool(name="mm", bufs=4, space="PSUM") as mm,
            tc.tile_pool(name="bp", bufs=2, space="PSUM") as bp,
        ):
            _psa = {"i": 0, "pend": None, "nobp": False}

            def _ps(name="ps"):
                _psa["i"] += 1
                if _psa["nobp"] or _psa["i"] % 4 in (1, 2):
                    return mm.tile([P, 512], f, tag="mm", name=name)
                if _psa["pend"] is None:
                    bt = bp.tile([P, 1024], f, tag="bp", name=name)
                    _psa["pend"] = bt
                    return bt[:, :512]
                bt = _psa["pend"]
                _psa["pend"] = None
                return bt[:, 512:]

            # ---------- resident activations ----------
            xT = ap_.tile([P, C, T], BF16, tag="xT")
            enhT = ap_.tile([P, C, T], fr, tag="enhT")
            recT = ap_.tile([P, C, T], BF16, tag="recT")
            ctxT = ap_.tile([P, NH, T], BF16, tag="ctxT")
            enhb = ap_.tile([P, C, T], BF16, tag="enhb")

            # ---------- phase 0: x loads first (sync q), consts on other qs
            ident = sg.tile([P, P], f, tag="ident")
            make_identity(nc, ident)
            xtoks = []
            xq = [nc.sync, nc.scalar]
            for i in range(TT):
                xtok = tp.tile([P, H], f, tag="t512", name=f"xtok{i}")
                xq[i % 2].dma_start(xtok, x_in[ts(i, P), :])
                xtoks.append(xtok)

            ones_bf = sg.tile([P, P], BF16, tag="onesbf")
            nc.scalar.dma_start(ones_bf, onesbf_in[:])
            ones_fr = sg.tile([P, P], fr, tag="onesfr")
            nc.scalar.dma_start(ones_fr, onesfr_in[:])
            ones8 = sg.tile([P, 2, P], FP8, tag="ones8")
            nc.scalar.dma_start(ones8, ones8_in[:])
            eps_t = sg.tile([P, 1], f, tag="eps")
            nc.vector.memset(eps_t, EPS)

            sb1 = sg.tile([P, C], f, tag="sb1")
            nc.scalar.dma_start(sb1, selb1[:])
            sw2 = sg.tile([P, C, O], BF16, tag="sw2")
            nc.scalar.dma_start(sw2, selW2a[:])
            sb2b = sg.tile([P, O], f, tag="sb2b")
            nc.scalar.dma_start(sb2b, bcast_ap(selb2, O))
            c1s = sg.tile([P, O, C], f, tag="c1s")
            nc.gpsimd.dma_start(c1s, c1_d[:])
            lngs = sg.tile([P, O, C], f, tag="lngs")
            nc.gpsimd.dma_start(lngs, lng_d[:])
            lnbs = sg.tile([P, O, C], f, tag="lnbs")
            nc.gpsimd.dma_start(lnbs, lnb_d[:])
            ibqk = sg.tile([P, 8], f, tag="ibqk")
            nc.gpsimd.dma_start(ibqk, inbqk[:])
            ibv = sg.tile([P, H], f, tag="ibv")
            nc.gpsimd.dma_start(ibv, bcast_ap(inbv, H))
            oba = sg.tile([P, C], f, tag="oba")
            nc.scalar.dma_start(oba, outb_a[:])
            rpb = sg.tile([P, C], f, tag="rpb")
            nc.scalar.dma_start(rpb, rpjb[:])
            dcs = sg.tile([P, D, C], f, tag="dcs")
            nc.scalar.dma_start(dcs, dc_d[:])
            gbs = sg.tile([P, C], f, tag="gbs")
            nc.gpsimd.dma_start(gbs, gateb[:])
            obF = sg.tile([P, H], f, tag="obF")
            nc.gpsimd.dma_start(obF, bcast_ap(outbF, H))

            def _xpose(lo, hi):
                for i in range(lo, hi):
                    for c in range(C):
                        tpsf = mm.tile([P, 512], f, tag="mm", name="tpsf")
                        tps = tpsf[:, :P]
                        nc.tensor.transpose(tps, xtoks[i][:, ts(c, P)], ident)
                        nc.scalar.copy(out=xT[:, c, ts(i, P)], in_=tps)

            _mark(nc, "router")
            # ---------- phase 1: router (interleaved with x transposes) ----
            sw1 = wstr.tile([P, C, H], BF16, tag="wstr")
            nc.sync.dma_start(sw1, selW1T[:])
            hrT = big.tile([P, C, T], BF16, tag="big")
            for t in range(TC):
                _xpose(t * 4, t * 4 + 4)
                for m in range(C):
                    ps = _ps()
                    for k in range(C):
                        nc.tensor.matmul(
                            ps, sw1[:, k, ts(m, P)], xT[:, k, ts(t, 512)],
                            start=(k == 0), stop=(k == C - 1),
                        )
                    nc.scalar.activation(
                        hrT[:, m, ts(t, 512)], ps, AF.Gelu,
                        bias=sb1[:, m : m + 1], scale=1.0,
                    )
            for i in range(TT):
                ps = _ps()
                for k in range(C):
                    nc.tensor.matmul(
                        ps[:, :O], hrT[:, k, ts(i, P)], sw2[:, k, :],
                        start=(k == 0), stop=(k == C - 1),
                    )
                lg = smp.tile([P, O], f, tag="sm6")
                nc.vector.tensor_tensor(lg, ps[:, :O], sb2b, ALU.add)
                ex = smp.tile([P, O], f, tag="sm6b")
                s_ = smp.tile([P, 1], f, tag="sm1")
                nc.scalar.activation(ex, lg, AF.Exp, accum_out=s_)
                nc.vector.reciprocal(s_, s_)
                opw = smp.tile([P, O], BF16, tag="sm6c")
                nc.vector.tensor_scalar_mul(opw, ex, s_)
                # transposing DMA store: SBUF (tok, o) order -> [O, T] layout
                nc.sync.dma_start(
                    bass.AP(tensor=opw_fea, offset=i * P, ap=[[1, P], [T, O]]),
                    opw,
                )

            # ------ phase 3: recursion, qc-pipelined across depth boundary --
            scale_s = 1.0 / math.sqrt(HD)
            kqs = [nc.sync, nc.scalar, nc.gpsimd]
            oqs = [nc.sync, nc.scalar, nc.gpsimd, nc.sync]
            projTs, qTs, ksl, vsl, wmap = {}, {}, {}, {}, {}

            def _w(key, dram):
                wt = wstr.tile([P, C, H], BF16, tag="wstr", name=key)
                nc.sync.dma_start(wt, dram[:])
                return wt

            def proj_t(d, t):
                srcT = enhb if d == 0 else recT
                for m in range(C):
                    ps = _ps()
                    for k in range(C):
                        nc.tensor.matmul(
                            ps, wmap["rw", d][:, k, ts(m, P)],
                            srcT[:, k, ts(t, 512)],
                            start=(k == 0), stop=(k == C - 1),
                        )
                    nc.vector.tensor_scalar_add(
                        projTs[d][:, m, ts(t, 512)], ps, rpb[:, m : m + 1]
                    )

            def kgemm_t(d, t):
                for moff in range(NH):
                    ps = _ps()
                    for k in range(C):
                        nc.tensor.matmul(
                            ps, wmap["wk", d][:, k, ts(moff, P)],
                            projTs[d][:, k, ts(t, 512)],
                            start=(k == 0), stop=(k == C - 1),
                        )
                    kc = tp.tile([P, 512], BF16, tag="b512",
                                 name=f"kc{d}_{t}_{moff}")
                    nc.vector.tensor_scalar_add(
                        kc, ps, ibqk[:, NH + moff : NH + moff + 1]
                    )
                    kqs[moff % 3].dma_start(k_send[t, :, moff, :], kc)

            def ag_k(d, t):
                if sim_mode:
                    nc.sync.dma_start(k_recv[t, 0], k_send[t])
                    nc.scalar.dma_start(k_recv[t, 1], k_send[t])
                else:
                    nc.gpsimd.collective_compute(
                        "AllGather", ALU.bypass, replica_groups=RG,
                        ins=[k_send[t]], outs=[k_recv[t]],
                    )

            def vgemm_t(d, t):
                for i2 in range(2 * t, 2 * t + 2):
                    vt = tp.tile([P, 1024], FP8, tag="v1k", bufs=2,
                                 name=f"vt{d}_{i2}")
                    for j in range(2):
                        i = 2 * i2 + j
                        ps = _ps()
                        for k in range(C):
                            nc.tensor.matmul(
                                ps, projTs[d][:, k, ts(i, P)],
                                wmap["wv", d][:, k, :],
                                start=(k == 0), stop=(k == C - 1),
                            )
                        nc.vector.tensor_tensor(
                            vt[:, ts(j, 512)], ps, ibv, ALU.add
                        )
                    kqs[i2 % 3].dma_start(
                        v_send[i2 // 2, :, 2 * (i2 % 2) : 2 * (i2 % 2) + 2, :],
                        vt.rearrange("p (a b) -> p a b", a=2),
                    )

            def ag_v(d, t):
                if sim_mode:
                    nc.sync.dma_start(v_recv[t, 0], v_send[t])
                    nc.scalar.dma_start(v_recv[t, 1], v_send[t])
                else:
                    nc.gpsimd.collective_compute(
                        "AllGather", ALU.bypass, replica_groups=RG,
                        ins=[v_send[t]], outs=[v_recv[t]],
                    )

            def kv_t(d, t):
                wmap["rw", d] = _w(f"rw{d}_{t}", rpjT)
                wmap["wk", d] = _w(f"wk{d}_{t}", inwkT)
                wmap["wv", d] = _w(f"wv{d}_{t}", inwvT)
                proj_t(d, t)
                kgemm_t(d, t)
                ag_k(d, t)
                vgemm_t(d, t)
                ag_v(d, t)

            kvf = {}

            def loads_t(d, t):
                if t == 0:
                    kvf["k"] = [
                        klp.tile([P, NH, TC, 512], BF16, tag="k", bufs=2,
                                 name=f"kf{s}") for s in range(2)
                    ]
                    kvf["v"] = [
                        klp.tile([P, TC, 4, H], FP8, tag="v", bufs=2,
                                 name=f"vf{s}") for s in range(2)
                    ]
                    kf = [x.rearrange("p h c b -> p h (c b)")
                          for x in kvf["k"]]
                    vf = [x.rearrange("p c j f -> p (c j) f")
                          for x in kvf["v"]]
                    for h in range(NH):
                        ksl[h] = [kf[0][:, h, :], kf[1][:, h, :]]
                        vsl[h] = [
                            vf[0][:, :, ts(h, P)], vf[1][:, :, ts(h, P)],
                        ]
                for s in range(2):
                    (nc.sync if s == 0 else nc.scalar).dma_start(
                        kvf["k"][s][:, :, t, :], k_recv[t, s]
                    )
                    (nc.gpsimd if s == 0 else nc.sync).dma_start(
                        kvf["v"][s][:, t, :, :], v_recv[t, s]
                    )

            def qgemm_t(d, t):
                if t == 0:
                    wmap["wq", d] = _w(f"wq{d}", inwqT)
                    for m in range(NH):
                        qTs[m] = klp.tile([P, T], BF16, tag="q",
                                          name=f"q{d}_{m}")
                for m in range(NH):
                    ps = _ps()
                    for k in range(C):
                        nc.tensor.matmul(
                            ps, wmap["wq", d][:, k, ts(m, P)],
                            projTs[d][:, k, ts(t, 512)],
                            start=(k == 0), stop=(k == C - 1),
                        )
                    if m % 2 == 0:
                        nc.scalar.activation(
                            qTs[m][:, ts(t, 512)], ps, AF.Identity,
                            bias=ibqk[:, m : m + 1], scale=1.0,
                        )
                    else:
                        nc.vector.tensor_scalar_add(
                            qTs[m][:, ts(t, 512)], ps, ibqk[:, m : m + 1]
                        )

            def att_qc(d, qc, heads=range(NH)):
                for h in heads:
                    ks, vs = ksl[h], vsl[h]
                    cps = mm.tile([P, 512], f, tag="mm", name="cps")
                    dacc = mm.tile([P, 512], f, tag="mm", name="dacc")
                    for idx, pj in enumerate((0, 1, 4, 5, 2, 3, 6, 7)):
                        slot, j2 = pj // 4, (pj % 4) * 2
                        ps2 = bp.tile([P, 1024], f, tag="bp", name="sc2")
                        nc.tensor.matmul(
                            ps2[:, :512], ks[slot][:, ts(j2, P)],
                            qTs[h][:, ts(qc, 512)], start=True, stop=True,
                        )
                        nc.tensor.matmul(
                            ps2[:, 512:], ks[slot][:, ts(j2 + 1, P)],
                            qTs[h][:, ts(qc, 512)], start=True, stop=True,
                        )
                        ex2 = xp.tile([P, 2, 512], FP8, tag="exp")
                        nc.scalar.activation(
                            ex2.rearrange("p a b -> p (a b)"), ps2,
                            AF.Exp, scale=scale_s,
                        )
                        nc.tensor.matmul(
                            cps, vs[slot][:, j2 : j2 + 2, :], ex2,
                            start=(idx == 0), stop=(idx == KT // 2 - 1),
                            perf_mode=DR,
                        )
                        nc.tensor.matmul(
                            dacc, ones8, ex2,
                            start=(idx == 0), stop=(idx == KT // 2 - 1),
                            perf_mode=DR,
                        )
                    rd = tp.tile([P, 512], f, tag="t512")
                    nc.vector.reciprocal(rd, dacc)
                    nc.vector.tensor_tensor(
                        ctxT[:, h, ts(qc, 512)], cps, rd, ALU.mult
                    )

            def outagg_t(d, t):
                attT = med.tile([P, C, 512], BF16, tag="attT")
                for m in range(C):
                    ps = _ps()
                    for k in range(C):
                        nc.tensor.matmul(
                            ps, wmap["ow", d][:, k, ts(m, P)],
                            ctxT[:, k, ts(t, 512)],
                            start=(k == 0), stop=(k == C - 1),
                        )
                    nc.vector.tensor_scalar_add(
                        attT[:, m, :], ps, oba[:, m : m + 1]
                    )
                for m in range(C):
                    ps = _ps()
                    for k in range(C):
                        nc.tensor.matmul(
                            ps, wmap["wa", d][:, k, ts(m, P)], attT[:, k, :],
                            start=(k == 0), stop=(k == C - 1),
                        )
                    nc.vector.tensor_scalar_add(
                        recT[:, m, ts(t, 512)], ps, dcs[:, d, m : m + 1]
                    )
                    sl = (slice(None), m, ts(t, 512))
                    nc.vector.scalar_tensor_tensor(
                        enhT[sl], recT[sl], 0.5 ** (d + 1), enhT[sl],
                        ALU.mult, ALU.add,
                    )

            def gate_t(t):
                for m in range(C):
                    nc.vector.tensor_copy(out=enhb[:, m, ts(t, 512)],
                                          in_=enhT[:, m, ts(t, 512)])
                for m in range(C):
                    ps = _ps()
                    for k in range(C):
                        nc.tensor.matmul(
                            ps, wmap["g1"][:, k, ts(m, P)], xT[:, k, ts(t, 512)],
                            start=(k == 0), stop=False,
                        )
                    for k in range(C):
                        nc.tensor.matmul(
                            ps, wmap["g2"][:, k, ts(m, P)],
                            enhb[:, k, ts(t, 512)],
                            start=False, stop=(k == C - 1),
                        )
                    gate = tp.tile([P, 512], f, tag="t512")
                    nc.scalar.activation(
                        gate, ps, AF.Sigmoid, bias=gbs[:, m : m + 1], scale=1.0,
                    )
                    sl = (slice(None), m, ts(t, 512))
                    d1 = tp.tile([P, 512], f, tag="t512")
                    nc.vector.tensor_tensor(d1, enhT[sl], xT[sl], ALU.subtract)
                    nc.vector.tensor_tensor(d1, d1, gate, ALU.mult)
                    nc.vector.tensor_tensor(enhT[sl], xT[sl], d1, ALU.add)

            def ln_final_t(t):
                pre_t = enhT[:, :, ts(t, 512)]
                mps = _ps()
                for k in range(C):
                    nc.tensor.matmul(mps, ones_fr, pre_t[:, k, :],
                                     start=(k == 0), stop=(k == C - 1))
                mean = tp.tile([P, 512], f, tag="t512")
                nc.scalar.activation(mean, mps, AF.Identity, scale=1.0 / H)
                sps = _ps()
                for k in range(C):
                    q = tp.tile([P, 512], fr, tag="t512", name=f"oq{k}_{t}")
                    if k % 2 == 0:
                        nc.vector.tensor_tensor(q, pre_t[:, k, :],
                                                pre_t[:, k, :], ALU.mult)
                    else:
                        nc.gpsimd.tensor_tensor(q, pre_t[:, k, :],
                                                pre_t[:, k, :], ALU.mult)
                    nc.tensor.matmul(sps, ones_fr, q,
                                     start=(k == 0), stop=(k == C - 1))
                e2 = tp.tile([P, 512], f, tag="t512")
                nc.scalar.activation(e2, sps, AF.Identity, scale=1.0 / H)
                m2 = tp.tile([P, 512], f, tag="t512")
                nc.vector.tensor_tensor(m2, mean, mean, ALU.mult)
                nc.vector.tensor_tensor(e2, e2, m2, ALU.subtract)
                nc.scalar.activation(e2, e2, AF.Sqrt, bias=eps_t, scale=1.0)
                rstd = tp.tile([P, 512], f, tag="t512")
                nc.vector.reciprocal(rstd, e2)
                lnt = med.tile([P, C, 512], BF16, tag="lnT")
                for m in range(C):
                    t1 = tp.tile([P, 512], f, tag="t512")
                    nc.vector.tensor_tensor(t1, pre_t[:, m, :], mean,
                                            ALU.subtract)
                    nc.vector.tensor_tensor(lnt[:, m, :], t1, rstd, ALU.mult)
                for ii in range(C):
                    i = t * C + ii
                    ps = _ps()
                    for k in range(C):
                        nc.tensor.matmul(
                            ps, lnt[:, k, ts(ii, P)], wmap["wf"][:, k, :],
                            start=(k == 0), stop=(k == C - 1),
                        )
                    ot = tp.tile([P, 512], f, tag="t512")
                    nc.vector.tensor_tensor(ot, ps, obF, ALU.add)
                    oqs[ii].dma_start(out_d[ts(i, P), :], ot)

            _mark(nc, "moe")
            # ---------- phase 2: MoE (6 experts) ----------
            # o-outer / t-inner; sqrt batched per 4 units to avoid ACT table
            # thrash; GEMM1 of batch b+1 emitted before applies of batch b.
            _psa["nobp"] = True
            units = [(o, t) for t in range(TC) for o in range(O)]
            batches = [units[i : i + 4] for i in range(0, len(units), 4)]
            w1s, w2s = {}, {}
            pres, wrps, means, rstds, vars_, accs = {}, {}, {}, {}, {}, {}

            def _moe_gemm1(o, t):
                w1 = wstr.tile([P, C, H], BF16, tag="wstr", name=f"w1_{o}_{t}")
                nc.sync.dma_start(w1, w1xT[o])
                w1s[o] = w1
                pre = prp.tile([P, C, 512], BF16, tag="pre", name=f"pre_{o}_{t}")
                for m in range(C):
                    ps = _ps(f"g1ps_{o}_{t}_{m}")
                    for k in range(C):
                        nc.tensor.matmul(
                            ps, w1s[o][:, k, ts(m, P)], xT[:, k, ts(t, 512)],
                            start=(k == 0), stop=(k == C - 1),
                        )
                    nc.scalar.activation(
                        pre[:, m, :], ps, AF.Identity,
                        bias=c1s[:, o, m : m + 1], scale=1.0,
                    )
                pres[(o, t)] = pre

            def _moe_stats(o, t):
                # chunk sums via accumulating ones-matmuls (PE); PSUM drains
                # to bf16 via ACT Identity (table-free); squares on DVE.
                pre = pres[(o, t)]
                mps = _ps(f"mps_{o}_{t}")
                for k in range(C):
                    nc.tensor.matmul(mps, ones_bf, pre[:, k, :],
                                     start=(k == 0), stop=(k == C - 1))
                mean = st.tile([P, 512], BF16, tag="mean", name=f"mean_{o}_{t}")
                nc.scalar.activation(mean, mps, AF.Identity, scale=1.0 / H)
                means[(o, t)] = mean
                sps = _ps(f"sps_{o}_{t}")
                for k in range(C):
                    q = tp.tile([P, 512], BF16, tag="b512", name=f"q{k}_{o}_{t}")
                    if k % 2 == 0:
                        nc.vector.tensor_tensor(q, pre[:, k, :], pre[:, k, :],
                                                ALU.mult)
                    else:
                        nc.gpsimd.tensor_tensor(q, pre[:, k, :], pre[:, k, :],
                                                ALU.mult)
                    nc.tensor.matmul(sps, ones_bf, q,
                                     start=(k == 0), stop=(k == C - 1))
                e2 = tp.tile([P, 512], BF16, tag="b512", name=f"e2_{o}_{t}")
                nc.scalar.activation(e2, sps, AF.Identity, scale=1.0 / H)
                m2 = tp.tile([P, 512], BF16, tag="b512", name=f"m2_{o}_{t}")
                nc.vector.tensor_tensor(m2, mean, mean, ALU.mult)
                var = st.tile([P, 512], BF16, tag="var", name=f"var_{o}_{t}")
                nc.vector.tensor_tensor(var, e2, m2, ALU.subtract)
                vars_[(o, t)] = var

            def _moe_rstd(batch):
                for (o, t) in batch:
                    sd = tp.tile([P, 512], BF16, tag="b512", name=f"sd_{o}_{t}")
                    nc.scalar.activation(
                        sd, vars_.pop((o, t)), AF.Sqrt, bias=eps_t, scale=1.0
                    )
                    rstd = st.tile([P, 512], BF16, tag="rstd", name=f"rs_{o}_{t}")
                    with nc.allow_low_precision(reason="bf16 LN rstd, 2e-2 gate"):
                        nc.vector.reciprocal(rstd, sd)
                    rstds[(o, t)] = rstd

            def _moe_apply(o, t):
                w2 = wstr.tile([P, C, H], BF16, tag="wstr", name=f"w2_{o}_{t}")
                nc.scalar.dma_start(w2, w2T[o])
                w2s[o] = w2
                pre = pres.pop((o, t))
                wrp = wrpp.tile([P, 512], BF16, tag="wrp", name=f"wrp_{o}_{t}")
                nc.sync.dma_start(
                    wrp,
                    bass.AP(
                        tensor=opw_fea, offset=o * T + t * 512,
                        ap=[[0, P], [1, 512]],
                    ),
                )
                mean = means.pop((o, t))
                rstd = rstds.pop((o, t))
                h = hp.tile([P, C, 512], BF16, tag="h", name=f"h_{o}_{t}")
                for m in range(C):
                    t1 = tp.tile([P, 512], BF16, tag="b512", name=f"n_{o}_{t}_{m}")
                    nc.vector.tensor_tensor(t1, pre[:, m, :], mean, ALU.subtract)
                    nc.vector.tensor_tensor(t1, t1, rstd, ALU.mult)
                    nc.scalar.activation(
                        h[:, m, :], t1, AF.Gelu,
                        bias=lnbs[:, o, m : m + 1], scale=lngs[:, o, m : m + 1],
                    )
                    # weight by router prob (per-token) before GEMM2
                    nc.vector.tensor_tensor(h[:, m, :], h[:, m, :], wrp, ALU.mult)
                if o == 0:
                    accs[t] = [
                        bp.tile([P, 1024], f, tag="bp", name=f"acc{t}a"),
                        bp.tile([P, 1024], f, tag="bp", name=f"acc{t}b"),
                    ]
                for m in range(C):
                    ps = accs[t][m // 2][:, ts(m % 2, 512)]
                    for k in range(C):
                        nc.tensor.matmul(
                            ps, w2s[o][:, k, ts(m, P)], h[:, k, :],
                            start=(o == 0 and k == 0),
                            stop=(o == O - 1 and k == C - 1),
                        )
                if o == O - 1:
                    for m in range(C):
                        sl = (slice(None), m, ts(t, 512))
                        acc = accs[t][m // 2][:, ts(m % 2, 512)]
                        nc.vector.tensor_tensor(enhT[sl], xT[sl], acc, ALU.add)
                        nc.vector.tensor_copy(out=enhb[sl], in_=enhT[sl])
                    _moe_t_done(t)

            def _moe_t_done(t):
                # launch the depth-0 K/V pipeline for this chunk while the
                # other chunk's MoE work continues
                _psa["nobp"] = False
                if t == 0:
                    projTs[0] = big.tile([P, C, T], BF16, tag="big",
                                         name="projT0")
                kv_t(0, t)
                loads_t(0, t)
                qgemm_t(0, t)
                _psa["nobp"] = t == 0

            for u in batches[0]:
                _moe_gemm1(*u)
            for bi, batch in enumerate(batches):
                for u in batch:
                    _moe_stats(*u)
                if bi + 1 < len(batches):
                    for u in batches[bi + 1]:
                        _moe_gemm1(*u)
                _moe_rstd(batch)
                for u in batch:
                    _moe_apply(*u)

            _mark(nc, "rec")
            # depth-0 kv/loads/q were launched from the MoE tail
            for d in range(D):
                wmap["ow", d] = _w(f"ow{d}", outwT)
                wmap["wa", d] = _w(f"wa{d}", waT)
                if d + 1 < D:
                    projTs[d + 1] = big.tile([P, C, T], BF16, tag="big",
                                             name=f"projT{d+1}")
                else:
                    wmap["g1"] = _w("g1", gw1T)
                    wmap["g2"] = _w("g2", gw2T)
                    wmap["wf"] = _w("wf", outwF)
                _mark(nc, f"d{d}:att")
                for h in range(NH):
                    for qc in range(TC):
                        att_qc(d, qc, heads=[h])
                for t in range(TC):
                    outagg_t(d, t)
                    if d + 1 < D:
                        kv_t(d + 1, t)
                        loads_t(d + 1, t)
                        qgemm_t(d + 1, t)
                    else:
                        gate_t(t)
            _mark(nc, "gate")
            for t in range(TC):
                ln_final_t(t)

    nc.compile()
    return nc


# ---------------------------------------------------------------------------
# host side
# ---------------------------------------------------------------------------

BF = ml_dtypes.bfloat16
F8NP = ml_dtypes.float8_e4m3


def _lhsT(w):
    """w [fout, fin] (y = x @ w.T) -> stationary layout [P, fin//P, fout]."""
    wt = np.ascontiguousarray(np.asarray(w, np.float32).T)
    fi, fo = wt.shape
    return np.ascontiguousarray(wt.reshape(fi // P, P, fo).transpose(1, 0, 2))


def _lhsT16(w):
    return _lhsT(w).astype(BF)


def _fvec(v, nch=None):
    v = np.asarray(v, np.float32)
    n = v.shape[-1] // P if nch is None else nch
    return np.ascontiguousarray(v.reshape(n, P).T)


def _prep_weights(i):
    w = {}
    w["selW1T"] = _lhsT16(i["sel_W1"])
    w["selW2a"] = _lhsT16(i["sel_W2"])
    w["selb1"] = _fvec(i["sel_b1"])
    w["selb2"] = np.asarray(i["sel_b2"], np.float32)
    w["w1xT"] = np.stack([_lhsT16(i["op_W1"][o, :, :H]) for o in range(O)])
    w["w2T"] = np.stack([_lhsT16(i["op_W2"][o]) for o in range(O)])
    c1 = np.stack(
        [i["op_emb"][o] @ i["op_W1"][o, :, H:].T + i["op_b1"][o] for o in range(O)]
    ).astype(np.float32)
    w["c1"] = np.ascontiguousarray(
        np.stack([_fvec(c1[o]) for o in range(O)]).transpose(1, 0, 2)
    )
    w["lng"] = np.ascontiguousarray(
        np.stack([_fvec(i["op_ln_g"][o]) for o in range(O)]).transpose(1, 0, 2)
    )
    w["lnb"] = np.ascontiguousarray(
        np.stack([_fvec(i["op_ln_b"][o]) for o in range(O)]).transpose(1, 0, 2)
    )
    w["inwqT"] = _lhsT16(i["attn_in_w"][:H])
    w["inwkT"] = _lhsT16(i["attn_in_w"][H : 2 * H])
    w["inwvT"] = _lhsT16(i["attn_in_w"][2 * H :])
    w["inbqk"] = _fvec(i["attn_in_b"][: 2 * H], 8)
    w["inbv"] = np.asarray(i["attn_in_b"][2 * H :], np.float32)
    w["outwT"] = _lhsT16(i["attn_out_w"])
    w["outb_a"] = _fvec(i["attn_out_b"])
    w["rpjT"] = _lhsT16(i["rec_proj_w"])
    w["rpjb"] = _fvec(i["rec_proj_b"])
    w["waT"] = _lhsT16(i["rec_agg_w"][:, :H])
    dc = np.stack(
        [
            i["depth_emb"][d] @ i["rec_agg_w"][:, H:].T + i["rec_agg_b"]
            for d in range(D)
        ]
    ).astype(np.float32)
    w["dc"] = np.ascontiguousarray(
        np.stack([_fvec(dc[d]) for d in range(D)]).transpose(1, 0, 2)
    )
    w["gw1T"] = _lhsT16(i["gate_w"][:, :H])
    w["gw2T"] = _lhsT16(i["gate_w"][:, H:])
    w["gateb"] = _fvec(i["gate_b"])
    # out LayerNorm gain folded into the final weight; bias into the final bias
    w["outwF"] = _lhsT16(np.asarray(i["out_w"]) * np.asarray(i["out_ln_g"])[None, :])
    w["outbF"] = (
        np.asarray(i["out_b"], np.float32)
        + np.asarray(i["out_ln_b"], np.float32) @ np.asarray(i["out_w"], np.float32).T
    )
    return w


def make_in_maps(inputs):
    inputs = {k: np.asarray(v, np.float32) for k, v in inputs.items()}
    hs = inputs["hidden_states"]
    w = _prep_weights(inputs)
    in_maps = []
    for c in range(8):
        b, half = c // 2, c % 2
        m = dict(w)
        m["onesbf_in"] = np.ones((P, P), BF)
        m["onesfr_in"] = np.ones((P, P), np.float32)
        m["ones8_in"] = np.ones((P, 2, P), F8NP)
        m["x"] = np.ascontiguousarray(hs[b, half * T : (half + 1) * T, :])
        in_maps.append(m)
    return in_maps


def assemble_out(results):
    out = np.empty((4, S, H), np.float32)
    for c in range(8):
        b, half = c // 2, c % 2
        out[b, half * T : (half + 1) * T, :] = results[c]["out"]
    return out


def kernel(**inputs):
    in_maps = make_in_maps(inputs)
    if "nc" not in _CACHE:
        _CACHE["nc"] = build_bass()
    res = run_bass_kernel_spmd(nc=_CACHE["nc"], in_maps=in_maps,
                               core_ids=list(range(8)))
    return assemble_out(res.results)


if __name__ == "__main__":
    print("build-only smoke test")
    build_bass()
    print("ok")


# revision 51
# speedup vs baseline: 1.0123x; 1.0123x over previous
"""Trainium2 Bass kernel for nn_LogicalReasoningLayer (moe_routing).

Sharding: 8 cores <- (batch b = c//2, seq half = c%2), 1024 tokens each.
Attention K/V exchanged between seq-half pairs via AllGather (groups of 2);
the gathered slots are already in canonical order, so both halves are read
straight out of k_recv/v_recv (no parity blending).

Activations feature-major [128p, C=4 chunks, T tokens]. Stationary matmul
operands (weights) in bf16; moving operands fp32r/bf16 (full-rate PE).
Attention AV + softmax denominator use fp8e4 DoubleRow matmuls (2 k-tiles
per instruction at 0.5 cycles/row).
"""

import sys

sys.path.insert(0, "/opt/trn_rl_repo")

import math

import ml_dtypes
import numpy as np

import concourse.bass as bass
import concourse.bacc as bacc
import concourse.tile as tile
from concourse import mybir
from concourse.bass import ts
from concourse.bass_utils import run_bass_kernel_spmd
from concourse.masks import make_identity

P = 128
H = 512
C = H // P          # 4 feature chunks
T = 1024            # tokens per core
TT = T // P         # 8 token tiles
TC = T // 512       # 2 token chunks (moving dim 512)
O = 6
NH = 4
HD = 128
D = 3
S = 2048
KT = S // P         # 16 key tiles
EPS = 1e-5
F32 = mybir.dt.float32
F32R = mybir.dt.float32r
BF16 = mybir.dt.bfloat16
FP8 = mybir.dt.float8e4
AF = mybir.ActivationFunctionType
ALU = mybir.AluOpType
DR = mybir.MatmulPerfMode.DoubleRow
RG = [[0, 1], [2, 3], [4, 5], [6, 7]]

_CACHE = {}
PHASES = []


def _mark(nc, label):
    PHASES.append((label, nc.get_next_instruction_name()))


def bcast_ap(handle, n_free):
    """[n_free] DRAM vector -> [P, n_free] stride-0 partition-broadcast AP."""
    return bass.AP(tensor=handle, offset=0, ap=[[0, P], [1, n_free]])


def build_bass(sim_mode=False):
    nc = bacc.Bacc("TRN2", target_bir_lowering=False, num_devices=8)

    f = F32
    fr = F32R
    x_in = nc.dram_tensor("x", [T, H], f, kind="ExternalInput")
    onesbf_in = nc.dram_tensor("onesbf_in", [P, P], BF16, kind="ExternalInput")
    onesfr_in = nc.dram_tensor("onesfr_in", [P, P], F32R, kind="ExternalInput")
    ones8_in = nc.dram_tensor("ones8_in", [P, 2, P], FP8, kind="ExternalInput")
    selW1T = nc.dram_tensor("selW1T", [P, C, H], BF16, kind="ExternalInput")
    selW2a = nc.dram_tensor("selW2a", [P, C, O], BF16, kind="ExternalInput")
    selb1 = nc.dram_tensor("selb1", [P, C], f, kind="ExternalInput")
    selb2 = nc.dram_tensor("selb2", [O], f, kind="ExternalInput")
    w1xT = nc.dram_tensor("w1xT", [O, P, C, H], BF16, kind="ExternalInput")
    w2T = nc.dram_tensor("w2T", [O, P, C, H], BF16, kind="ExternalInput")
    c1_d = nc.dram_tensor("c1", [P, O, C], f, kind="ExternalInput")
    lng_d = nc.dram_tensor("lng", [P, O, C], f, kind="ExternalInput")
    lnb_d = nc.dram_tensor("lnb", [P, O, C], f, kind="ExternalInput")
    inwqT = nc.dram_tensor("inwqT", [P, C, H], BF16, kind="ExternalInput")
    inwkT = nc.dram_tensor("inwkT", [P, C, H], BF16, kind="ExternalInput")
    inwvT = nc.dram_tensor("inwvT", [P, C, H], BF16, kind="ExternalInput")
    inbqk = nc.dram_tensor("inbqk", [P, 8], f, kind="ExternalInput")
    inbv = nc.dram_tensor("inbv", [H], f, kind="ExternalInput")
    outwT = nc.dram_tensor("outwT", [P, C, H], BF16, kind="ExternalInput")
    outb_a = nc.dram_tensor("outb_a", [P, C], f, kind="ExternalInput")
    rpjT = nc.dram_tensor("rpjT", [P, C, H], BF16, kind="ExternalInput")
    rpjb = nc.dram_tensor("rpjb", [P, C], f, kind="ExternalInput")
    waT = nc.dram_tensor("waT", [P, C, H], BF16, kind="ExternalInput")
    dc_d = nc.dram_tensor("dc", [P, D, C], f, kind="ExternalInput")
    gw1T = nc.dram_tensor("gw1T", [P, C, H], BF16, kind="ExternalInput")
    gw2T = nc.dram_tensor("gw2T", [P, C, H], BF16, kind="ExternalInput")
    gateb = nc.dram_tensor("gateb", [P, C], f, kind="ExternalInput")
    outwF = nc.dram_tensor("outwF", [P, C, H], BF16, kind="ExternalInput")
    outbF = nc.dram_tensor("outbF", [H], f, kind="ExternalInput")

    out_d = nc.dram_tensor("out", [T, H], f, kind="ExternalOutput")

    opw_fea = nc.dram_tensor("opw_fea", [O, T], BF16)
    k_send = nc.dram_tensor("k_send", [TC, P, NH, 512], BF16)
    v_send = nc.dram_tensor("v_send", [TC, P, 4, H], FP8)
    k_recv = nc.dram_tensor("k_recv", [TC, 2, P, NH, 512], BF16)
    v_recv = nc.dram_tensor("v_recv", [TC, 2, P, 4, H], FP8)

    with tile.TileContext(nc) as tc:
        with (
            tc.tile_pool(name="singles", bufs=1) as sg,
            tc.tile_pool(name="act", bufs=1) as ap_,
            tc.tile_pool(name="big", bufs=2) as big,
            tc.tile_pool(name="med", bufs=2) as med,
            tc.tile_pool(name="wstr", bufs=5) as wstr,
            tc.tile_pool(name="t512", bufs=4) as tp,
            tc.tile_pool(name="st", bufs=5) as st,
            tc.tile_pool(name="prep", bufs=6) as prp,
            tc.tile_pool(name="hp", bufs=2) as hp,
            tc.tile_pool(name="wrp", bufs=2) as wrpp,
            tc.tile_pool(name="kloc", bufs=4) as klp,
            tc.tile_pool(name="exp", bufs=4) as xp,
            tc.tile_pool(name="sm", bufs=4) as smp,
            tc.tile_pool(name="mm", bufs=4, space="PSUM") as mm,
            tc.tile_pool(name="bp", bufs=2, space="PSUM") as bp,
        ):
            _psa = {"i": 0, "pend": None, "nobp": False}

            def _ps(name="ps"):
                _psa["i"] += 1
                if _psa["nobp"] or _psa["i"] % 4 in (1, 2):
                    return mm.tile([P, 512], f, tag="mm", name=name)
                if _psa["pend"] is None:
                    bt = bp.tile([P, 1024], f, tag="bp", name=name)
                    _psa["pend"] = bt
                    return bt[:, :512]
                bt = _psa["pend"]
                _psa["pend"] = None
                return bt[:, 512:]

            # ---------- resident activations ----------
            xT = ap_.tile([P, C, T], BF16, tag="xT")
            enhT = ap_.tile([P, C, T], fr, tag="enhT")
            recT = ap_.tile([P, C, T], BF16, tag="recT")
            ctxT = ap_.tile([P, NH, T], BF16, tag="ctxT")
            enhb = ap_.tile([P, C, T], BF16, tag="enhb")

            # ---------- phase 0: x loads first (sync q), consts on other qs
            ident = sg.tile([P, P], f, tag="ident")
            make_identity(nc, ident)
            xtoks = []
            xq = [nc.sync, nc.scalar]
            for i in range(TT):
                xtok = tp.tile([P, H], f, tag="t512", name=f"xtok{i}")
                xq[i % 2].dma_start(xtok, x_in[ts(i, P), :])
                xtoks.append(xtok)

            ones_bf = sg.tile([P, P], BF16, tag="onesbf")
            nc.gpsimd.dma_start(ones_bf, onesbf_in[:])
            ones_fr = sg.tile([P, P], fr, tag="onesfr")
            nc.gpsimd.dma_start(ones_fr, onesfr_in[:])
            ones8 = sg.tile([P, 2, P], FP8, tag="ones8")
            nc.gpsimd.dma_start(ones8, ones8_in[:])
            eps_t = sg.tile([P, 1], f, tag="eps")
            nc.vector.memset(eps_t, EPS)

            sb1 = sg.tile([P, C], f, tag="sb1")
            nc.gpsimd.dma_start(sb1, selb1[:])
            sw2 = sg.tile([P, C, O], BF16, tag="sw2")
            nc.gpsimd.dma_start(sw2, selW2a[:])
            sb2b = sg.tile([P, O], f, tag="sb2b")
            nc.gpsimd.dma_start(sb2b, bcast_ap(selb2, O))
            c1s = sg.tile([P, O, C], f, tag="c1s")
            nc.gpsimd.dma_start(c1s, c1_d[:])
            lngs = sg.tile([P, O, C], f, tag="lngs")
            nc.gpsimd.dma_start(lngs, lng_d[:])
            lnbs = sg.tile([P, O, C], f, tag="lnbs")
            nc.gpsimd.dma_start(lnbs, lnb_d[:])
            ibqk = sg.tile([P, 8], f, tag="ibqk")
            nc.gpsimd.dma_start(ibqk, inbqk[:])
            ibv = sg.tile([P, H], f, tag="ibv")
            nc.gpsimd.dma_start(ibv, bcast_ap(inbv, H))
            oba = sg.tile([P, C], f, tag="oba")
            nc.gpsimd.dma_start(oba, outb_a[:])
            rpb = sg.tile([P, C], f, tag="rpb")
            nc.gpsimd.dma_start(rpb, rpjb[:])
            dcs = sg.tile([P, D, C], f, tag="dcs")
            nc.gpsimd.dma_start(dcs, dc_d[:])
            gbs = sg.tile([P, C], f, tag="gbs")
            nc.gpsimd.dma_start(gbs, gateb[:])
            obF = sg.tile([P, H], f, tag="obF")
            nc.gpsimd.dma_start(obF, bcast_ap(outbF, H))

            def _xpose(lo, hi):
                for i in range(lo, hi):
                    for c in range(C):
                        tpsf = mm.tile([P, 512], f, tag="mm", name="tpsf")
                        tps = tpsf[:, :P]
                        nc.tensor.transpose(tps, xtoks[i][:, ts(c, P)], ident)
                        nc.scalar.copy(out=xT[:, c, ts(i, P)], in_=tps)

            _mark(nc, "router")
            # ---------- phase 1: router (interleaved with x transposes) ----
            sw1 = wstr.tile([P, C, H], BF16, tag="wstr")
            nc.sync.dma_start(sw1, selW1T[:])
            hrT = big.tile([P, C, T], BF16, tag="big")
            for t in range(TC):
                _xpose(t * 4, t * 4 + 4)
                for m in range(C):
                    ps = _ps()
                    for k in range(C):
                        nc.tensor.matmul(
                            ps, sw1[:, k, ts(m, P)], xT[:, k, ts(t, 512)],
                            start=(k == 0), stop=(k == C - 1),
                        )
                    nc.scalar.activation(
                        hrT[:, m, ts(t, 512)], ps, AF.Gelu,
                        bias=sb1[:, m : m + 1], scale=1.0,
                    )
            for i in range(TT):
                ps = _ps()
                for k in range(C):
                    nc.tensor.matmul(
                        ps[:, :O], hrT[:, k, ts(i, P)], sw2[:, k, :],
                        start=(k == 0), stop=(k == C - 1),
                    )
                lg = smp.tile([P, O], f, tag="sm6")
                nc.vector.tensor_tensor(lg, ps[:, :O], sb2b, ALU.add)
                ex = smp.tile([P, O], f, tag="sm6b")
                s_ = smp.tile([P, 1], f, tag="sm1")
                nc.scalar.activation(ex, lg, AF.Exp, accum_out=s_)
                nc.vector.reciprocal(s_, s_)
                opw = smp.tile([P, O], BF16, tag="sm6c")
                nc.vector.tensor_scalar_mul(opw, ex, s_)
                # transposing DMA store: SBUF (tok, o) order -> [O, T] layout
                nc.sync.dma_start(
                    bass.AP(tensor=opw_fea, offset=i * P, ap=[[1, P], [T, O]]),
                    opw,
                )

            # ------ phase 3: recursion, qc-pipelined across depth boundary --
            scale_s = 1.0 / math.sqrt(HD)
            kqs = [nc.sync, nc.scalar, nc.gpsimd]
            oqs = [nc.sync, nc.scalar, nc.gpsimd, nc.sync]
            projTs, qTs, ksl, vsl, wmap = {}, {}, {}, {}, {}

            def _w(key, dram):
                wt = wstr.tile([P, C, H], BF16, tag="wstr", name=key)
                nc.sync.dma_start(wt, dram[:])
                return wt

            def proj_t(d, t):
                srcT = enhb if d == 0 else recT
                for m in range(C):
                    ps = _ps()
                    for k in range(C):
                        nc.tensor.matmul(
                            ps, wmap["rw", d][:, k, ts(m, P)],
                            srcT[:, k, ts(t, 512)],
                            start=(k == 0), stop=(k == C - 1),
                        )
                    nc.vector.tensor_scalar_add(
                        projTs[d][:, m, ts(t, 512)], ps, rpb[:, m : m + 1]
                    )

            def kgemm_t(d, t):
                for moff in range(NH):
                    ps = _ps()
                    for k in range(C):
                        nc.tensor.matmul(
                            ps, wmap["wk", d][:, k, ts(moff, P)],
                            projTs[d][:, k, ts(t, 512)],
                            start=(k == 0), stop=(k == C - 1),
                        )
                    kc = tp.tile([P, 512], BF16, tag="b512",
                                 name=f"kc{d}_{t}_{moff}")
                    nc.vector.tensor_scalar_add(
                        kc, ps, ibqk[:, NH + moff : NH + moff + 1]
                    )
                    kqs[moff % 3].dma_start(k_send[t, :, moff, :], kc)

            def ag_k(d, t):
                if sim_mode:
                    nc.sync.dma_start(k_recv[t, 0], k_send[t])
                    nc.scalar.dma_start(k_recv[t, 1], k_send[t])
                else:
                    nc.gpsimd.collective_compute(
                        "AllGather", ALU.bypass, replica_groups=RG,
                        ins=[k_send[t]], outs=[k_recv[t]],
                    )

            def vgemm_t(d, t):
                for i2 in range(2 * t, 2 * t + 2):
                    vt = tp.tile([P, 1024], FP8, tag="v1k", bufs=2,
                                 name=f"vt{d}_{i2}")
                    for j in range(2):
                        i = 2 * i2 + j
                        ps = _ps()
                        for k in range(C):
                            nc.tensor.matmul(
                                ps, projTs[d][:, k, ts(i, P)],
                                wmap["wv", d][:, k, :],
                                start=(k == 0), stop=(k == C - 1),
                            )
                        nc.vector.tensor_tensor(
                            vt[:, ts(j, 512)], ps, ibv, ALU.add
                        )
                    kqs[i2 % 3].dma_start(
                        v_send[i2 // 2, :, 2 * (i2 % 2) : 2 * (i2 % 2) + 2, :],
                        vt.rearrange("p (a b) -> p a b", a=2),
                    )

            def ag_v(d, t):
                if sim_mode:
                    nc.sync.dma_start(v_recv[t, 0], v_send[t])
                    nc.scalar.dma_start(v_recv[t, 1], v_send[t])
                else:
                    nc.gpsimd.collective_compute(
                        "AllGather", ALU.bypass, replica_groups=RG,
                        ins=[v_send[t]], outs=[v_recv[t]],
                    )

            def kv_t(d, t):
                wmap["rw", d] = _w(f"rw{d}_{t}", rpjT)
                wmap["wk", d] = _w(f"wk{d}_{t}", inwkT)
                wmap["wv", d] = _w(f"wv{d}_{t}", inwvT)
                proj_t(d, t)
                kgemm_t(d, t)
                ag_k(d, t)
                vgemm_t(d, t)
                ag_v(d, t)

            kvf = {}

            def loads_t(d, t):
                if t == 0:
                    kvf["k"] = [
                        klp.tile([P, NH, TC, 512], BF16, tag="k", bufs=2,
                                 name=f"kf{s}") for s in range(2)
                    ]
                    kvf["v"] = [
                        klp.tile([P, TC, 4, H], FP8, tag="v", bufs=2,
                                 name=f"vf{s}") for s in range(2)
                    ]
                    kf = [x.rearrange("p h c b -> p h (c b)")
                          for x in kvf["k"]]
                    vf = [x.rearrange("p c j f -> p (c j) f")
                          for x in kvf["v"]]
                    for h in range(NH):
                        ksl[h] = [kf[0][:, h, :], kf[1][:, h, :]]
                        vsl[h] = [
                            vf[0][:, :, ts(h, P)], vf[1][:, :, ts(h, P)],
                        ]
                for s in range(2):
                    (nc.sync if s == 0 else nc.scalar).dma_start(
                        kvf["k"][s][:, :, t, :], k_recv[t, s]
                    )
                    (nc.gpsimd if s == 0 else nc.sync).dma_start(
                        kvf["v"][s][:, t, :, :], v_recv[t, s]
                    )

            def qgemm_t(d, t):
                if t == 0:
                    wmap["wq", d] = _w(f"wq{d}", inwqT)
                    for m in range(NH):
                        qTs[m] = klp.tile([P, T], BF16, tag="q",
                                          name=f"q{d}_{m}")
                for m in range(NH):
                    ps = _ps()
                    for k in range(C):
                        nc.tensor.matmul(
                            ps, wmap["wq", d][:, k, ts(m, P)],
                            projTs[d][:, k, ts(t, 512)],
                            start=(k == 0), stop=(k == C - 1),
                        )
                    if m % 2 == 0:
                        nc.scalar.activation(
                            qTs[m][:, ts(t, 512)], ps, AF.Identity,
                            bias=ibqk[:, m : m + 1], scale=1.0,
                        )
                    else:
                        nc.vector.tensor_scalar_add(
                            qTs[m][:, ts(t, 512)], ps, ibqk[:, m : m + 1]
                        )

            def att_qc(d, qc, heads=range(NH)):
                for h in heads:
                    ks, vs = ksl[h], vsl[h]
                    cps = mm.tile([P, 512], f, tag="mm", name="cps")
                    dacc = mm.tile([P, 512], f, tag="mm", name="dacc")
                    for idx, pj in enumerate((0, 1, 4, 5, 2, 3, 6, 7)):
                        slot, j2 = pj // 4, (pj % 4) * 2
                        ps2 = bp.tile([P, 1024], f, tag="bp", name="sc2")
                        nc.tensor.matmul(
                            ps2[:, :512], ks[slot][:, ts(j2, P)],
                            qTs[h][:, ts(qc, 512)], start=True, stop=True,
                        )
                        nc.tensor.matmul(
                            ps2[:, 512:], ks[slot][:, ts(j2 + 1, P)],
                            qTs[h][:, ts(qc, 512)], start=True, stop=True,
                        )
                        ex2 = xp.tile([P, 2, 512], FP8, tag="exp")
                        nc.scalar.activation(
                            ex2.rearrange("p a b -> p (a b)"), ps2,
                            AF.Exp, scale=scale_s,
                        )
                        nc.tensor.matmul(
                            cps, vs[slot][:, j2 : j2 + 2, :], ex2,
                            start=(idx == 0), stop=(idx == KT // 2 - 1),
                            perf_mode=DR,
                        )
                        nc.tensor.matmul(
                            dacc, ones8, ex2,
                            start=(idx == 0), stop=(idx == KT // 2 - 1),
                            perf_mode=DR,
                        )
                    rd = tp.tile([P, 512], f, tag="t512")
                    nc.vector.reciprocal(rd, dacc)
                    nc.vector.tensor_tensor(
                        ctxT[:, h, ts(qc, 512)], cps, rd, ALU.mult
                    )

            def outagg_t(d, t):
                attT = med.tile([P, C, 512], BF16, tag="attT")
                for m in range(C):
                    ps = _ps()
                    for k in range(C):
                        nc.tensor.matmul(
                            ps, wmap["ow", d][:, k, ts(m, P)],
                            ctxT[:, k, ts(t, 512)],
                            start=(k == 0), stop=(k == C - 1),
                        )
                    nc.vector.tensor_scalar_add(
                        attT[:, m, :], ps, oba[:, m : m + 1]
                    )
                for m in range(C):
                    ps = _ps()
                    for k in range(C):
                        nc.tensor.matmul(
                            ps, wmap["wa", d][:, k, ts(m, P)], attT[:, k, :],
                            start=(k == 0), stop=(k == C - 1),
                        )
                    nc.vector.tensor_scalar_add(
                        recT[:, m, ts(t, 512)], ps, dcs[:, d, m : m + 1]
                    )
                    sl = (slice(None), m, ts(t, 512))
                    nc.vector.scalar_tensor_tensor(
                        enhT[sl], recT[sl], 0.5 ** (d + 1), enhT[sl],
                        ALU.mult, ALU.add,
                    )

            def gate_t(t):
                for m in range(C):
                    nc.vector.tensor_copy(out=enhb[:, m, ts(t, 512)],
                                          in_=enhT[:, m, ts(t, 512)])
                for m in range(C):
                    ps = _ps()
                    for k in range(C):
                        nc.tensor.matmul(
                            ps, wmap["g1"][:, k, ts(m, P)], xT[:, k, ts(t, 512)],
                            start=(k == 0), stop=False,
                        )
                    for k in range(C):
                        nc.tensor.matmul(
                            ps, wmap["g2"][:, k, ts(m, P)],
                            enhb[:, k, ts(t, 512)],
                            start=False, stop=(k == C - 1),
                        )
                    gate = tp.tile([P, 512], f, tag="t512")
                    nc.scalar.activation(
                        gate, ps, AF.Sigmoid, bias=gbs[:, m : m + 1], scale=1.0,
                    )
                    sl = (slice(None), m, ts(t, 512))
                    d1 = tp.tile([P, 512], f, tag="t512")
                    nc.vector.tensor_tensor(d1, enhT[sl], xT[sl], ALU.subtract)
                    nc.vector.tensor_tensor(d1, d1, gate, ALU.mult)
                    nc.vector.tensor_tensor(enhT[sl], xT[sl], d1, ALU.add)

            def ln_final_t(t):
                pre_t = enhT[:, :, ts(t, 512)]
                mps = _ps()
                for k in range(C):
                    nc.tensor.matmul(mps, ones_fr, pre_t[:, k, :],
                                     start=(k == 0), stop=(k == C - 1))
                mean = tp.tile([P, 512], f, tag="t512")
                nc.scalar.activation(mean, mps, AF.Identity, scale=1.0 / H)
                sps = _ps()
                for k in range(C):
                    q = tp.tile([P, 512], fr, tag="t512", name=f"oq{k}_{t}")
                    if k % 2 == 0:
                        nc.vector.tensor_tensor(q, pre_t[:, k, :],
                                                pre_t[:, k, :], ALU.mult)
                    else:
                        nc.gpsimd.tensor_tensor(q, pre_t[:, k, :],
                                                pre_t[:, k, :], ALU.mult)
                    nc.tensor.matmul(sps, ones_fr, q,
                                     start=(k == 0), stop=(k == C - 1))
                e2 = tp.tile([P, 512], f, tag="t512")
                nc.scalar.activation(e2, sps, AF.Identity, scale=1.0 / H)
                m2 = tp.tile([P, 512], f, tag="t512")
                nc.vector.tensor_tensor(m2, mean, mean, ALU.mult)
                nc.vector.tensor_tensor(e2, e2, m2, ALU.subtract)
                nc.scalar.activation(e2, e2, AF.Sqrt, bias=eps_t, scale=1.0)
                rstd = tp.tile([P, 512], f, tag="t512")
                nc.vector.reciprocal(rstd, e2)
                lnt = med.tile([P, C, 512], BF16, tag="lnT")
                for m in range(C):
                    t1 = tp.tile([P, 512], f, tag="t512")
                    nc.vector.tensor_tensor(t1, pre_t[:, m, :], mean,
                                            ALU.subtract)
                    nc.vector.tensor_tensor(lnt[:, m, :], t1, rstd, ALU.mult)
                for ii in range(C):
                    i = t * C + ii
                    ps = _ps()
                    for k in range(C):
                        nc.tensor.matmul(
                            ps, lnt[:, k, ts(ii, P)], wmap["wf"][:, k, :],
                            start=(k == 0), stop=(k == C - 1),
                        )
                    ot = tp.tile([P, 512], f, tag="t512")
                    nc.vector.tensor_tensor(ot, ps, obF, ALU.add)
                    oqs[ii].dma_start(out_d[ts(i, P), :], ot)

            _mark(nc, "moe")
            # ---------- phase 2: MoE (6 experts) ----------
            # o-outer / t-inner; sqrt batched per 4 units to avoid ACT table
            # thrash; GEMM1 of batch b+1 emitted before applies of batch b.
            _psa["nobp"] = True
            units = [(o, t) for t in range(TC) for o in range(O)]
            batches = [units[i : i + 4] for i in range(0, len(units), 4)]
            w1s, w2s = {}, {}
            pres, wrps, means, rstds, vars_, accs = {}, {}, {}, {}, {}, {}

            def _moe_gemm1(o, t):
                w1 = wstr.tile([P, C, H], BF16, tag="wstr", name=f"w1_{o}_{t}")
                nc.sync.dma_start(w1, w1xT[o])
                w1s[o] = w1
                pre = prp.tile([P, C, 512], BF16, tag="pre", name=f"pre_{o}_{t}")
                for m in range(C):
                    ps = _ps(f"g1ps_{o}_{t}_{m}")
                    for k in range(C):
                        nc.tensor.matmul(
                            ps, w1s[o][:, k, ts(m, P)], xT[:, k, ts(t, 512)],
                            start=(k == 0), stop=(k == C - 1),
                        )
                    nc.scalar.activation(
                        pre[:, m, :], ps, AF.Identity,
                        bias=c1s[:, o, m : m + 1], scale=1.0,
                    )
                pres[(o, t)] = pre

            def _moe_stats(o, t):
                # chunk sums via accumulating ones-matmuls (PE); PSUM drains
                # to bf16 via ACT Identity (table-free); squares on DVE.
                pre = pres[(o, t)]
                mps = _ps(f"mps_{o}_{t}")
                for k in range(C):
                    nc.tensor.matmul(mps, ones_bf, pre[:, k, :],
                                     start=(k == 0), stop=(k == C - 1))
                mean = st.tile([P, 512], BF16, tag="mean", name=f"mean_{o}_{t}")
                nc.scalar.activation(mean, mps, AF.Identity, scale=1.0 / H)
                means[(o, t)] = mean
                sps = _ps(f"sps_{o}_{t}")
                for k in range(C):
                    q = tp.tile([P, 512], BF16, tag="b512", name=f"q{k}_{o}_{t}")
                    if k % 2 == 0:
                        nc.vector.tensor_tensor(q, pre[:, k, :], pre[:, k, :],
                                                ALU.mult)
                    else:
                        nc.gpsimd.tensor_tensor(q, pre[:, k, :], pre[:, k, :],
                                                ALU.mult)
                    nc.tensor.matmul(sps, ones_bf, q,
                                     start=(k == 0), stop=(k == C - 1))
                e2 = tp.tile([P, 512], BF16, tag="b512", name=f"e2_{o}_{t}")
                nc.scalar.activation(e2, sps, AF.Identity, scale=1.0 / H)
                m2 = tp.tile([P, 512], BF16, tag="b512", name=f"m2_{o}_{t}")
                nc.vector.tensor_tensor(m2, mean, mean, ALU.mult)
                var = st.tile([P, 512], BF16, tag="var", name=f"var_{o}_{t}")
                nc.vector.tensor_tensor(var, e2, m2, ALU.subtract)
                vars_[(o, t)] = var

            def _moe_rstd(batch):
                for (o, t) in batch:
                    sd = tp.tile([P, 512], BF16, tag="b512", name=f"sd_{o}_{t}")
                    nc.scalar.activation(
                        sd, vars_.pop((o, t)), AF.Sqrt, bias=eps_t, scale=1.0
                    )
                    rstd = st.tile([P, 512], BF16, tag="rstd", name=f"rs_{o}_{t}")
                    with nc.allow_low_precision(reason="bf16 LN rstd, 2e-2 gate"):
                        nc.vector.reciprocal(rstd, sd)
                    rstds[(o, t)] = rstd

            def _moe_apply(o, t):
                w2 = wstr.tile([P, C, H], BF16, tag="wstr", name=f"w2_{o}_{t}")
                nc.scalar.dma_start(w2, w2T[o])
                w2s[o] = w2
                pre = pres.pop((o, t))
                wrp = wrpp.tile([P, 512], BF16, tag="wrp", name=f"wrp_{o}_{t}")
                nc.sync.dma_start(
                    wrp,
                    bass.AP(
                        tensor=opw_fea, offset=o * T + t * 512,
                        ap=[[0, P], [1, 512]],
                    ),
                )
                mean = means.pop((o, t))
                rstd = rstds.pop((o, t))
                h = hp.tile([P, C, 512], BF16, tag="h", name=f"h_{o}_{t}")
                for m in range(C):
                    t1 = tp.tile([P, 512], BF16, tag="b512", name=f"n_{o}_{t}_{m}")
                    nc.vector.tensor_tensor(t1, pre[:, m, :], mean, ALU.subtract)
                    nc.vector.tensor_tensor(t1, t1, rstd, ALU.mult)
                    nc.scalar.activation(
                        h[:, m, :], t1, AF.Gelu,
                        bias=lnbs[:, o, m : m + 1], scale=lngs[:, o, m : m + 1],
                    )
                    # weight by router prob (per-token) before GEMM2
                    nc.vector.tensor_tensor(h[:, m, :], h[:, m, :], wrp, ALU.mult)
                if o == 0:
                    accs[t] = [
                        bp.tile([P, 1024], f, tag="bp", name=f"acc{t}a"),
                        bp.tile([P, 1024], f, tag="bp", name=f"acc{t}b"),
                    ]
                for m in range(C):
                    ps = accs[t][m // 2][:, ts(m % 2, 512)]
                    for k in range(C):
                        nc.tensor.matmul(
                            ps, w2s[o][:, k, ts(m, P)], h[:, k, :],
                            start=(o == 0 and k == 0),
                            stop=(o == O - 1 and k == C - 1),
                        )
                if o == O - 1:
                    for m in range(C):
                        sl = (slice(None), m, ts(t, 512))
                        acc = accs[t][m // 2][:, ts(m % 2, 512)]
                        nc.vector.tensor_tensor(enhT[sl], xT[sl], acc, ALU.add)
                        nc.vector.tensor_copy(out=enhb[sl], in_=enhT[sl])
                    _moe_t_done(t)

            def _moe_t_done(t):
                # launch the depth-0 K/V pipeline for this chunk while the
                # other chunk's MoE work continues
                _psa["nobp"] = False
                if t == 0:
                    projTs[0] = big.tile([P, C, T], BF16, tag="big",
                                         name="projT0")
                kv_t(0, t)
                loads_t(0, t)
                qgemm_t(0, t)
                _psa["nobp"] = t == 0

            for u in batches[0]:
                _moe_gemm1(*u)
            for bi, batch in enumerate(batches):
                for u in batch:
                    _moe_stats(*u)
                if bi + 1 < len(batches):
                    for u in batches[bi + 1]:
                        _moe_gemm1(*u)
                _moe_rstd(batch)
                for u in batch:
                    _moe_apply(*u)

            _mark(nc, "rec")
            # depth-0 kv/loads/q were launched from the MoE tail
            for d in range(D):
                wmap["ow", d] = _w(f"ow{d}", outwT)
                wmap["wa", d] = _w(f"wa{d}", waT)
                if d + 1 < D:
                    projTs[d + 1] = big.tile([P, C, T], BF16, tag="big",
                                             name=f"projT{d+1}")
                else:
                    wmap["g1"] = _w("g1", gw1T)
                    wmap["g2"] = _w("g2", gw2T)
                    wmap["wf"] = _w("wf", outwF)
                _mark(nc, f"d{d}:att")
                for h in range(NH):
                    for qc in range(TC):
                        att_qc(d, qc, heads=[h])
                for t in range(TC):
                    outagg_t(d, t)
                    if d + 1 < D:
                        kv_t(d + 1, t)
                        loads_t(d + 1, t)
                        qgemm_t(d + 1, t)
                    else:
                        gate_t(t)
            _mark(nc, "gate")
            for t in range(TC):
                ln_final_t(t)

    nc.compile()
    return nc


# ---------------------------------------------------------------------------
# host side
# ---------------------------------------------------------------------------

BF = ml_dtypes.bfloat16
F8NP = ml_dtypes.float8_e4m3


def _lhsT(w):
    """w [fout, fin] (y = x @ w.T) -> stationary layout [P, fin//P, fout]."""
    wt = np.ascontiguousarray(np.asarray(w, np.float32).T)
    fi, fo = wt.shape
    return np.ascontiguousarray(wt.reshape(fi // P, P, fo).transpose(1, 0, 2))


def _lhsT16(w):
    return _lhsT(w).astype(BF)


def _fvec(v, nch=None):
    v = np.asarray(v, np.float32)
    n = v.shape[-1] // P if nch is None else nch
    return np.ascontiguousarray(v.reshape(n, P).T)


def _prep_weights(i):
    w = {}
    w["selW1T"] = _lhsT16(i["sel_W1"])
    w["selW2a"] = _lhsT16(i["sel_W2"])
    w["selb1"] = _fvec(i["sel_b1"])
    w["selb2"] = np.asarray(i["sel_b2"], np.float32)
    w["w1xT"] = np.stack([_lhsT16(i["op_W1"][o, :, :H]) for o in range(O)])
    w["w2T"] = np.stack([_lhsT16(i["op_W2"][o]) for o in range(O)])
    c1 = np.stack(
        [i["op_emb"][o] @ i["op_W1"][o, :, H:].T + i["op_b1"][o] for o in range(O)]
    ).astype(np.float32)
    w["c1"] = np.ascontiguousarray(
        np.stack([_fvec(c1[o]) for o in range(O)]).transpose(1, 0, 2)
    )
    w["lng"] = np.ascontiguousarray(
        np.stack([_fvec(i["op_ln_g"][o]) for o in range(O)]).transpose(1, 0, 2)
    )
    w["lnb"] = np.ascontiguousarray(
        np.stack([_fvec(i["op_ln_b"][o]) for o in range(O)]).transpose(1, 0, 2)
    )
    w["inwqT"] = _lhsT16(i["attn_in_w"][:H])
    w["inwkT"] = _lhsT16(i["attn_in_w"][H : 2 * H])
    w["inwvT"] = _lhsT16(i["attn_in_w"][2 * H :])
    w["inbqk"] = _fvec(i["attn_in_b"][: 2 * H], 8)
    w["inbv"] = np.asarray(i["attn_in_b"][2 * H :], np.float32)
    w["outwT"] = _lhsT16(i["attn_out_w"])
    w["outb_a"] = _fvec(i["attn_out_b"])
    w["rpjT"] = _lhsT16(i["rec_proj_w"])
    w["rpjb"] = _fvec(i["rec_proj_b"])
    w["waT"] = _lhsT16(i["rec_agg_w"][:, :H])
    dc = np.stack(
        [
            i["depth_emb"][d] @ i["rec_agg_w"][:, H:].T + i["rec_agg_b"]
            for d in range(D)
        ]
    ).astype(np.float32)
    w["dc"] = np.ascontiguousarray(
        np.stack([_fvec(dc[d]) for d in range(D)]).transpose(1, 0, 2)
    )
    w["gw1T"] = _lhsT16(i["gate_w"][:, :H])
    w["gw2T"] = _lhsT16(i["gate_w"][:, H:])
    w["gateb"] = _fvec(i["gate_b"])
    # out LayerNorm gain folded into the final weight; bias into the final bias
    w["outwF"] = _lhsT16(np.asarray(i["out_w"]) * np.asarray(i["out_ln_g"])[None, :])
    w["outbF"] = (
        np.asarray(i["out_b"], np.float32)
        + np.asarray(i["out_ln_b"], np.float32) @ np.asarray(i["out_w"], np.float32).T
    )
    return w


def make_in_maps(inputs):
    inputs = {k: np.asarray(v, np.float32) for k, v in inputs.items()}
    hs = inputs["hidden_states"]
    w = _prep_weights(inputs)
    in_maps = []
    for c in range(8):
        b, half = c // 2, c % 2
        m = dict(w)
        m["onesbf_in"] = np.ones((P, P), BF)
        m["onesfr_in"] = np.ones((P, P), np.float32)
        m["ones8_in"] = np.ones((P, 2, P), F8NP)
        m["x"] = np.ascontiguousarray(hs[b, half * T : (half + 1) * T, :])
        in_maps.append(m)
    return in_maps


def assemble_out(results):
    out = np.empty((4, S, H), np.float32)
    for c in range(8):
        b, half = c // 2, c % 2
        out[b, half * T : (half + 1) * T, :] = results[c]["out"]
    return out


def kernel(**inputs):
    in_maps = make_in_maps(inputs)
    if "nc" not in _CACHE:
        _CACHE["nc"] = build_bass()
    res = run_bass_kernel_spmd(nc=_CACHE["nc"], in_maps=in_maps,
                               core_ids=list(range(8)))
    return assemble_out(res.results)


if __name__ == "__main__":
    print("build-only smoke test")
    build_bass()
    print("ok")
